# revision 13
# baseline (speedup 1.0000x reference)
"""CapsLayer2D dynamic-routing kernel for 8x TRN2 NeuronCores — v7.

Shapes (hardcoded):
  inputs: [B=16, R=8, C=8, I=128, DIN=16] fp32
  W:      [K=32, I=128, DIN=16, DOUT=16] fp32
  out:    [B, R, C, K, DOUT] fp32

Math: 3-round dynamic routing, closed form:
  U[p,k] = res (I x O);  s0 = mean_i U_i;  A = U^T U
  y1 = A s0 = U^T(U s0) ; y2 = A y1
  g = factor(s0); s1 = s0 + g*y1; f = factor(s1)
  out = factor(s2)*s2,  s2 = s0 + (g+f)*y1 + f*g*y2
  factor(s) = (|s|^2/(1+|s|^2)) / sqrt(|s|^2+eps)

v7 design (batch sharded across 8 cores, W replicated):
  W is pre-ordered (g, o, k8) per chunk so production psum columns land
  directly in the res layout: res per k-group g is (i128, o16, k8).
  Both routing contractions run on the PE as identity-stationary matmul
  streams over CONTIGUOUS moving slices:
    q[i,k] = sum_o uv[o,i,k] -> 16 accumulating mms over 256-col o-slices
    y[o,k] = sum_i ut[i,o,k] -> 32 mms per quarter over 128-col i-slices
  DVE does only the two broadcast multiplies per (group, quarter), all
  in 2x_1p fp16 mode (k8-last layouts). The v4 DVE o-tree is gone.

  Pipeline: units = (round, group, quarter), TWO-deep software pipeline
  (ut/yred of unit n emitted in unit n+2) so the uv->ore->qev->ut chain
  latency hides under two uv ops. UA/UT share one 3-slot sbuf tag.

  Production: group-PAIRED 256-col mms (rhs spans g,g+1), [P,2048] quads
  with the band b -> bank b rule, 8 mms + 2x 1024-col evacs per quad.
  Groups 0,1 (16 quads) run up front interleaved with the s0 chain
  (which uses a 'py'-tag psum so the quad tag never stalls); groups 2,3
  interleave 2 quads per routing unit. Post-compile, redundant
  Ldweights with identical stationary operands are elided.
"""

import sys

import numpy as np

sys.path.insert(0, "/opt/trn_rl_repo")

P, I, D, D2, K, O = 128, 128, 16, 32, 32, 16
KC = 8          # k-group size
NG = K // KC    # 4 groups
GN = I * KC * O  # 16384 elements per group block
KO = K * O      # 512
QI = 32         # i's per quarter
NQ = I // QI    # 4 quarters
N_CORES = 8
EPS = 1e-7

_PROGRAM = None


def _build_program():
    from contextlib import ExitStack

    import concourse.tile as tile
    from concourse import bacc, mybir

    F32 = mybir.dt.float32
    F16 = mybir.dt.float16
    ADD = mybir.AluOpType.add
    X = mybir.AxisListType.X
    SQRT = mybir.ActivationFunctionType.Sqrt
    COPY = mybir.ActivationFunctionType.Copy

    nc = bacc.Bacc("TRN2", target_bir_lowering=False, debug=False)

    xt_d = nc.dram_tensor("xt", [P, 32 * 128], F16, kind="ExternalInput").ap()
    wr_d = nc.dram_tensor("wr", [P, 32 * KO], F16, kind="ExternalInput").ap()
    ei_d = nc.dram_tensor("ei", [P, 256], F16, kind="ExternalInput").ap()
    out_d = nc.dram_tensor("out", [P, KO], F32, kind="ExternalOutput").ap()

    with ExitStack() as ctx:
        tc = ctx.enter_context(tile.TileContext(nc))

        pp = ctx.enter_context(tc.tile_pool(name="pp", bufs=1, space="PSUM"))
        rp = ctx.enter_context(tc.tile_pool(name="resp", bufs=1))
        sm = ctx.enter_context(tc.tile_pool(name="small", bufs=1))

        res = rp.tile([P, NG * GN], F16)     # [P, 65536] per-g (i, o, k8)
        AUX = rp.tile([P, 256], F16)         # [E | E/64]
        Xt = rp.tile([P, 32 * 128], F16)
        Wr = rp.tile([P, 32 * KO], F16)      # per-chunk cols (g, o, k8)
        E = AUX[:, 0:128]
        E64 = AUX[:, 128:256]

        # ---- small tiles ----
        s0h = sm.tile([P, KO], F16, tag="s0h")   # (g, o, k8)
        y1h = sm.tile([P, KO], F16, tag="y1h")   # holds y1/64
        y2h = sm.tile([P, KO], F16, tag="y2h")   # holds y2/64
        sqb = sm.tile([P, KO], F32, tag="sqb")
        s2f = sm.tile([P, KO], F32, tag="s2f")
        eps_t = sm.tile([P, 1], F32, tag="eps")
        nc.vector.memset(eps_t[:], EPS)

        def factor(src, out32, tag):
            """out32[p, (g,k8)] = (nsq/(1+nsq))/sqrt(nsq+eps), nsq over o."""
            nc.scalar.square(sqb[:], src)
            nsq = sm.tile([P, K], F32, tag=f"nsq_{tag}")
            nc.vector.tensor_reduce(
                nsq[:].rearrange("p (g k) -> p g k", g=NG),
                sqb[:].rearrange("p (g o k) -> p g k o", g=NG, o=O),
                X, ADD,
            )
            rt = sm.tile([P, K], F32, tag="f_rt")
            nc.scalar.activation(rt[:], nsq[:], SQRT, bias=eps_t[:])
            b1 = sm.tile([P, K], F32, tag="f_b1")
            nc.scalar.add(b1[:], nsq[:], 1.0)
            den = sm.tile([P, K], F32, tag="f_den")
            nc.vector.tensor_mul(den[:], rt[:], b1[:])
            rin = sm.tile([P, K], F32, tag="f_rin")
            nc.vector.reciprocal(rin[:], den[:])
            nc.vector.tensor_mul(out32[:], nsq[:], rin[:])

        def bcast_ok(v32):
            """[p, (g,k8)] -> broadcast view [p, g, o, k8] (k8 last, 2x)."""
            return (
                v32[:].rearrange("p (g k) -> p g k", g=NG)
                .unsqueeze(2).broadcast_to([P, NG, O, KC])
            )

        def vKO(t):
            return t[:].rearrange("p (g o k) -> p g o k", g=NG, o=O)

        nc.sync.dma_start(AUX[:], ei_d)
        for q in range(2):
            nc.sync.dma_start(
                Xt[:, q * 2048:(q + 1) * 2048], xt_d[:, q * 2048:(q + 1) * 2048]
            )
        for q in range(8):
            nc.sync.dma_start(
                Wr[:, q * 2048:(q + 1) * 2048], wr_d[:, q * 2048:(q + 1) * 2048]
            )

        # ---- PE warm-up (clock ramp; overlaps DMA) ----
        q0 = pp.tile([P, 2048], F32, tag="quad")
        for w in range(24):
            nc.tensor.matmul(
                q0[:, 1024:1152],
                Xt[0:32, 0:128],
                Xt[0:32, 0:128],
                start=(w == 0),
                stop=(w == 23),
                tile_position=(0, 0),
            )

        # ---- production: group-paired quads ----
        # quad (gp, cq): 8 banded 256-col mms; c = 2*cq + j (j 0..1),
        # i = 8*cq + 4*j + b; psum col = b*512 + j*256 + gl*128 + n,
        # g = 2*gp + gl. Two 1024-col evacs (one per gl).
        def produce_quad(gp, cq):
            qt = pp.tile([P, 2048], F32, tag="quad")
            for b in range(4):
                r0 = b * 32
                for j in range(2):
                    c = 2 * cq + j
                    nc.tensor.matmul(
                        qt[:, b * 512 + j * 256:b * 512 + (j + 1) * 256],
                        Xt[r0:r0 + 32, c * 128:(c + 1) * 128],
                        Wr[r0:r0 + 32, c * KO + gp * 256:c * KO + (gp + 1) * 256],
                        start=True,
                        stop=True,
                        tile_position=(r0, 0),
                    )
            srcv = qt[:].rearrange(
                "p (b j gl n) -> p gl j b n", b=4, j=2, gl=2
            )
            with nc.allow_low_precision(reason="res fp16"):
                for gl in range(2):
                    g = 2 * gp + gl
                    dstv = (
                        res[:, g * GN:(g + 1) * GN]
                        .rearrange("p (i n) -> p i n", i=I)
                        [:, 8 * cq:8 * (cq + 1), :]
                        .rearrange("p (j b) n -> p j b n", j=2, b=4)
                    )
                    nc.scalar.copy(dstv, srcv[:, gl])

        # ---- s0 chain (full-depth X.W sum over i), on the 'py' psum tag ----
        ps0 = pp.tile([P, 512], F32, tag="py", bufs=2)

        def s0_mms(c0, c1):
            for c in range(c0, c1):
                nc.tensor.matmul(
                    ps0[:],
                    Xt[:, c * 128:(c + 1) * 128],
                    Wr[:, c * KO:(c + 1) * KO],
                    start=(c == 0),
                    stop=(c == 31),
                )

        # ---- routing unit pieces ----
        def uv_op(v_h16, g, q):
            UA = rp.tile([P, QI * KC * O], F16, tag="uat", bufs=3, name="UA")
            rg = (
                res[:, g * GN:(g + 1) * GN]
                .rearrange("p (i o k) -> p i o k", i=I, o=O)
                [:, q * QI:(q + 1) * QI, :, :]
                .transpose([0, 2, 1, 3])
            )
            vg = (
                v_h16[:, g * KC * O:(g + 1) * KC * O]
                .rearrange("p (o k) -> p o k", o=O)
                .unsqueeze(2).broadcast_to([P, O, QI, KC])
            )
            nc.vector.tensor_mul(
                UA[:].rearrange("p (o i k) -> p o i k", o=O, i=QI), rg, vg
            )
            return UA

        def ore_op(UA):
            pq = pp.tile([P, 256], F32, tag="pq", bufs=2)
            for o in range(O):
                nc.tensor.matmul(
                    pq[:], E, UA[:, o * 256:(o + 1) * 256],
                    start=(o == 0), stop=(o == O - 1),
                )
            return pq

        def qev_op(pq):
            qt = rp.tile([P, QI * KC], F16, tag="qt", bufs=3)
            with nc.allow_low_precision(reason="q fp16"):
                nc.scalar.copy(qt[:], pq[:])
            return qt

        def ut_op(g, q, qt):
            UT = rp.tile([P, QI * KC * O], F16, tag="uat", bufs=3, name="UT")
            rg = (
                res[:, g * GN:(g + 1) * GN]
                .rearrange("p (i o k) -> p i o k", i=I, o=O)
                [:, q * QI:(q + 1) * QI, :, :]
            )
            qb = (
                qt[:].rearrange("p (i k) -> p i k", i=QI)
                .unsqueeze(2).broadcast_to([P, QI, O, KC])
            )
            nc.vector.tensor_mul(
                UT[:].rearrange("p (i o k) -> p i o k", i=QI, o=O), rg, qb
            )
            return UT

        def yred_op(UT, py, e_acc, q):
            for j in range(QI):
                nc.tensor.matmul(
                    py[:, 0:128], e_acc, UT[:, j * 128:(j + 1) * 128],
                    start=(q == 0 and j == 0),
                    stop=(q == NQ - 1 and j == QI - 1),
                )

        def ytail_op(py, y_out16, g):
            with nc.allow_low_precision(reason="y fp16"):
                nc.scalar.copy(
                    y_out16[:, g * KC * O:(g + 1) * KC * O], py[:, 0:128]
                )

        # ---- startup: g0+g1 production interleaved with s0 chain ----
        for cq in range(2):
            produce_quad(0, cq)
        s0_mms(0, 8)
        produce_quad(0, 2)
        s0_mms(8, 16)
        produce_quad(0, 3)
        s0_mms(16, 24)
        produce_quad(0, 4)
        s0_mms(24, 32)
        with nc.allow_low_precision(reason="s0 fp16"):
            nc.scalar.activation(s0h[:], ps0[:], COPY, scale=1.0 / I)
        for cq in range(5, 16):
            produce_quad(0, cq)

        g32 = sm.tile([P, K], F32, tag="g32")
        g64 = sm.tile([P, K], F32, tag="g64")
        f32_ = sm.tile([P, K], F32, tag="f32_")
        gf = sm.tile([P, K], F32, tag="gf")
        fg = sm.tile([P, K], F32, tag="fg")
        h32 = sm.tile([P, K], F32, tag="h32")
        outf = sm.tile([P, KO], F32, tag="outf")
        s1f = outf
        SC = 64.0

        # 64 global units: (round, group, quarter)
        units = [
            (rnd, g, q) for rnd in range(2) for g in range(NG) for q in range(NQ)
        ]
        # production interleave for groups 2,3: 2 quads per unit, units 0..7
        prod_sched = {u: [(1, 2 * u), (1, 2 * u + 1)] for u in range(8)}

        pys = {}
        pend = []  # (UT, py, g, q, rnd) depth-2 queue

        def flush_one(y1h=y1h, y2h=y2h):
            pUT, ppy, pg_, pq_, prnd = pend.pop(0)
            yred_op(pUT, ppy, E64 if prnd == 0 else E, pq_)
            if pq_ == NQ - 1:
                ytail_op(ppy, y1h if prnd == 0 else y2h, pg_)
                if prnd == 1:
                    # fold this group's y2 term into s2 right away
                    gsl = slice(pg_ * 128, (pg_ + 1) * 128)
                    nc.vector.tensor_mul(
                        sqb[:, gsl].rearrange("p (o k) -> p o k", o=O),
                        y2h[:, gsl].rearrange("p (o k) -> p o k", o=O),
                        fg[:].rearrange("p (g k) -> p g k", g=NG)
                        [:, pg_].unsqueeze(1).broadcast_to([P, O, KC]),
                    )
                    nc.vector.tensor_add(
                        s2f[:, gsl], s2f[:, gsl], sqb[:, gsl]
                    )

        with nc.allow_low_precision(reason="fp16 routing"):
            for idx, (rnd, g, q) in enumerate(units):
                if q == 0:
                    pys[(rnd, g)] = pp.tile(
                        [P, 128], F32, tag="py", bufs=2, name="py"
                    )
                py = pys[(rnd, g)]
                UA = uv_op(s0h if rnd == 0 else y1h, g, q)
                pq = ore_op(UA)
                qt = qev_op(pq)
                for pg, cq in prod_sched.get(idx, ()):
                    produce_quad(pg, cq)
                if len(pend) >= 2:
                    flush_one()
                UT = ut_op(g, q, qt)
                pend.append((UT, py, g, q, rnd))
                # factor-chain injections (run during round 2's stream)
                if idx == 18:
                    factor(s0h[:], g32, "g")
                    nc.scalar.mul(g64[:], g32[:], SC)
                if idx == 19:
                    # s1 = s0 + g*y1 = s0 + (64 g)*(y1/64)
                    nc.vector.tensor_mul(vKO(s1f), vKO(y1h), bcast_ok(g64))
                    nc.vector.tensor_add(s1f[:], s1f[:], s0h[:])
                    factor(s1f[:], f32_, "f")
                if idx == 20:
                    nc.vector.tensor_add(gf[:], g32[:], f32_[:])
                    nc.scalar.mul(gf[:], gf[:], SC)
                    nc.vector.tensor_mul(fg[:], f32_[:], g32[:])
                    nc.scalar.mul(fg[:], fg[:], SC)
                    nc.vector.tensor_mul(vKO(s2f), vKO(y1h), bcast_ok(gf))
                    nc.vector.tensor_add(s2f[:], s2f[:], s0h[:])
            while pend:
                flush_one()

            # out = factor(s2)*s2  (y2 terms already folded per group)
            factor(s2f[:], h32, "h")
            nc.vector.tensor_mul(vKO(outf), vKO(s2f), bcast_ok(h32))
        nc.sync.dma_start(out_d, outf[:])

    nc.compile()
    _elide_redundant_ldweights(nc)
    return nc


def _elide_redundant_ldweights(nc):
    """Remove Ldweights whose stationary AP + tile_position match the
    immediately preceding Ldweights on the PE queue (no other PE inst in
    between). Skips any carrying sem waits/updates."""
    import concourse.mybir as mybir

    removed = 0
    for block in nc.m.functions[0].blocks:
        insts = list(block.instructions)
        prev_key = None
        rm = set()
        for inst in insts:
            if inst.engine != mybir.EngineType.PE:
                continue
            op = inst.concise_opcode()
            if op == "Ldweights":
                k = (str(inst.ins[0]), str(getattr(inst, "tile_position", None)))
                if k == prev_key and not inst.has_wait() and not inst.has_update():
                    rm.add(id(inst))
                prev_key = k
            elif op == "Matmult":
                continue
            else:
                prev_key = None
        if rm:
            block.instructions = [i for i in insts if id(i) not in rm]
            removed += len(rm)
    return removed


def _host_prep(x, W):
    """x: [B,R,C,I,D] f32; W: [K,I,D,O] f32 -> per-core Xt + shared W_r.

    Xt[(i%4)*32+d, (i//4)*128+p] = x[p, i, d] (d < 16, pad to 32).
    W_r[(i%4)*32+d, (i//4)*512 + g*128 + o*8 + k8] = W[g*8+k8, i, d, o].
    """
    xs = x.reshape(N_CORES, P, I, D)
    a = xs.transpose(0, 2, 3, 1).reshape(N_CORES, 32, 4, D, P)
    ap = np.zeros((N_CORES, 32, 4, D2, P), np.float32)
    ap[:, :, :, 0:D, :] = a
    xt = (
        ap.transpose(0, 2, 3, 1, 4)
        .reshape(N_CORES, 128, 32 * 128)
        .astype(np.float16)
    )
    # (g, o, k8)-ordered W columns
    wko = (
        W.reshape(NG, KC, I, D, O)
        .transpose(2, 3, 0, 4, 1)
        .reshape(I, D, KO)
    )
    b = wko.reshape(32, 4, D, KO)
    bp = np.zeros((32, 4, D2, KO), np.float32)
    bp[:, :, 0:D, :] = b
    wr = bp.transpose(1, 2, 0, 3).reshape(128, 32 * KO).astype(np.float16)
    return xt, wr


def _aux_host():
    e = np.eye(128, dtype=np.float16)
    return np.concatenate([e, e / 64.0], axis=1).astype(np.float16)


def _get_program():
    global _PROGRAM
    if _PROGRAM is None:
        _PROGRAM = _build_program()
    return _PROGRAM


def _in_maps(x, W):
    xt, wr = _host_prep(x, W)
    ei = _aux_host()
    return [
        {"xt": np.ascontiguousarray(xt[c]), "wr": wr, "ei": ei}
        for c in range(N_CORES)
    ]


def _unpack(raw):
    """raw: [P, 512] f32 with cols (g4, o16, k8) -> [2, 8, 8, K, O]."""
    a = raw.reshape(2, 8, 8, NG, O, KC)
    return np.ascontiguousarray(
        a.transpose(0, 1, 2, 3, 5, 4).reshape(2, 8, 8, K, O)
    )


def kernel(**inputs):
    x = np.ascontiguousarray(np.asarray(inputs["inputs"], dtype=np.float32))
    W = np.ascontiguousarray(np.asarray(inputs["W"], dtype=np.float32))
    assert x.shape == (16, 8, 8, 128, 16) and W.shape == (32, 128, 16, 16)

    from concourse.bass_utils import run_bass_kernel_spmd

    nc = _get_program()
    in_maps = _in_maps(x, W)
    r = run_bass_kernel_spmd(nc, in_maps, list(range(N_CORES)))
    outs = [_unpack(r.results[c]["out"]) for c in range(N_CORES)]
    return np.concatenate(outs, axis=0).astype(np.float32)


# revision 20
# speedup vs baseline: 1.2505x; 1.2505x over previous
"""CapsLayer2D dynamic-routing kernel for 8x TRN2 NeuronCores — v9.

Shapes (hardcoded):
  inputs: [B=16, R=8, C=8, I=128, DIN=16] fp32
  W:      [K=32, I=128, DIN=16, DOUT=16] fp32
  out:    [B, R, C, K, DOUT] fp32

Math: 3-round dynamic routing, closed form:
  U[p,k] = res (I x O);  s0 = mean_i U_i;  A = U^T U
  y1 = A s0 = U^T(U s0) ; y2 = A y1
  g = factor(s0); s1 = s0 + g*y1; f = factor(s1)
  out = factor(s2)*s2,  s2 = s0 + (g+f)*y1 + f*g*y2
  factor(s) = (|s|^2/(1+|s|^2)) / sqrt(|s|^2+eps)

v9 design (batch sharded across 8 cores, W replicated):
  W is pre-ordered (g, o, k8) per chunk so production psum columns land
  directly in the res layout: res per k-group g is (i128, o16, k8).
  Both routing contractions run on the PE as identity-stationary matmul
  streams over CONTIGUOUS moving slices (strided moving ops are 2x
  slower on the PE):
    q[i,k] = sum_o uv[o,i,k] -> 16 accumulating mms over 256-col o-slices
    y[o,k] = sum_i ut[i,o,k] -> 32 mms per quarter over 128-col i-slices
  DVE does only the two broadcast multiplies per (group, quarter), all
  in 2x_1p fp16 mode (k8-last layouts incl. the v/q broadcasts). The
  old DVE o-tree (~100us) is gone.

  Pipeline: 64 global units = (round, group, quarter), TWO-deep software
  pipeline (ut/yred of unit n emitted in unit n+2) so the
  uv->ore->qev->ut chain latency hides under two uv ops. UA/UT share a
  3-slot sbuf tag.

  Production: [P,2048] psum quads, 16 banded mms each, psum col =
  b*512 + j*128 so 32-row band b writes only psum bank b — bands
  sharing a bank is an NRT_EXEC_UNIT_UNRECOVERABLE crash on HW.
  Quad tag is single-buffered (psum budget: 4 + pq 2 + py 2 = 8 banks);
  routing PE work interleaves between quads to hide the evac waits.
"""

import sys

import numpy as np

sys.path.insert(0, "/opt/trn_rl_repo")

P, I, D, D2, K, O = 128, 128, 16, 32, 32, 16
KC = 8          # k-group size
NG = K // KC    # 4 groups
GN = I * KC * O  # 16384 elements per group block
KO = K * O      # 512
QI = 32         # i's per quarter
NQ = I // QI    # 4 quarters
N_CORES = 8
EPS = 1e-7

_PROGRAM = None


def _build_program():
    from contextlib import ExitStack

    import concourse.tile as tile
    from concourse import bacc, mybir

    F32 = mybir.dt.float32
    F16 = mybir.dt.float16
    ADD = mybir.AluOpType.add
    X = mybir.AxisListType.X
    SQRT = mybir.ActivationFunctionType.Sqrt
    COPY = mybir.ActivationFunctionType.Copy

    nc = bacc.Bacc("TRN2", target_bir_lowering=False, debug=False)

    xt_d = nc.dram_tensor("xt", [P, 32 * 128], F16, kind="ExternalInput").ap()
    wr_d = nc.dram_tensor("wr", [P, 32 * KO], F16, kind="ExternalInput").ap()
    ei_d = nc.dram_tensor("ei", [P, 256], F16, kind="ExternalInput").ap()
    out_d = nc.dram_tensor("out", [P, KO], F32, kind="ExternalOutput").ap()

    with ExitStack() as ctx:
        tc = ctx.enter_context(tile.TileContext(nc))

        pp = ctx.enter_context(tc.tile_pool(name="pp", bufs=1, space="PSUM"))
        rp = ctx.enter_context(tc.tile_pool(name="resp", bufs=1))
        sm = ctx.enter_context(tc.tile_pool(name="small", bufs=1))

        res = rp.tile([P, NG * GN], F16)     # [P, 65536] per-g (i, o, k8)
        AUX = rp.tile([P, 256], F16)         # [E | E/64]
        Xt = rp.tile([P, 32 * 128], F16)
        Wr = rp.tile([P, 32 * KO], F16)      # per-chunk cols (g, o, k8)
        E = AUX[:, 0:128]
        E64 = AUX[:, 128:256]

        # ---- small tiles ----
        s0h = sm.tile([P, KO], F16, tag="s0h")   # (g, o, k8)
        y1h = sm.tile([P, KO], F16, tag="y1h")   # holds y1/64
        y2h = sm.tile([P, KO], F16, tag="y2h")   # holds y2/64
        sqb = sm.tile([P, KO], F32, tag="sqb")
        s2f = sm.tile([P, KO], F32, tag="s2f")
        eps_t = sm.tile([P, 1], F32, tag="eps")
        nc.vector.memset(eps_t[:], EPS)

        def factor(src, out32, tag):
            """out32[p, (g,k8)] = (nsq/(1+nsq))/sqrt(nsq+eps), nsq over o."""
            nc.scalar.square(sqb[:], src)
            nsq = sm.tile([P, K], F32, tag=f"nsq_{tag}")
            nc.vector.tensor_reduce(
                nsq[:].rearrange("p (g k) -> p g k", g=NG),
                sqb[:].rearrange("p (g o k) -> p g k o", g=NG, o=O),
                X, ADD,
            )
            rt = sm.tile([P, K], F32, tag="f_rt")
            nc.scalar.activation(rt[:], nsq[:], SQRT, bias=eps_t[:])
            b1 = sm.tile([P, K], F32, tag="f_b1")
            nc.scalar.add(b1[:], nsq[:], 1.0)
            den = sm.tile([P, K], F32, tag="f_den")
            nc.vector.tensor_mul(den[:], rt[:], b1[:])
            rin = sm.tile([P, K], F32, tag="f_rin")
            nc.vector.reciprocal(rin[:], den[:])
            nc.vector.tensor_mul(out32[:], nsq[:], rin[:])

        def bcast_ok(v32):
            """[p, (g,k8)] -> broadcast view [p, g, o, k8] (k8 last, 2x)."""
            return (
                v32[:].rearrange("p (g k) -> p g k", g=NG)
                .unsqueeze(2).broadcast_to([P, NG, O, KC])
            )

        def vKO(t):
            return t[:].rearrange("p (g o k) -> p g o k", g=NG, o=O)

        nc.sync.dma_start(AUX[:], ei_d)
        for q in range(2):
            nc.sync.dma_start(
                Xt[:, q * 2048:(q + 1) * 2048], xt_d[:, q * 2048:(q + 1) * 2048]
            )
        for q in range(8):
            nc.sync.dma_start(
                Wr[:, q * 2048:(q + 1) * 2048], wr_d[:, q * 2048:(q + 1) * 2048]
            )

        # ---- PE warm-up (clock ramp; overlaps DMA) ----
        q0 = pp.tile([P, 2048], F32, tag="quad")
        for w in range(24):
            nc.tensor.matmul(
                q0[:, 1024:1152],
                Xt[0:32, 0:128],
                Xt[0:32, 0:128],
                start=(w == 0),
                stop=(w == 23),
                tile_position=(0, 0),
            )

        # ---- production: [P,2048] quads, 16 banded mms, bank b = band b --
        # i = 16*cq + 4*j + b ; c = 4*cq + j ; psum col = b*512 + j*128.
        # Evac as two 1024-col halves with dims (j2, b4, n128).
        def produce_quad(g, cq):
            qt = pp.tile([P, 2048], F32, tag="quad")
            for b in range(4):
                r0 = b * 32
                for j in range(4):
                    c = 4 * cq + j
                    nc.tensor.matmul(
                        qt[:, b * 512 + j * 128:b * 512 + (j + 1) * 128],
                        Xt[r0:r0 + 32, c * 128:(c + 1) * 128],
                        Wr[r0:r0 + 32, c * KO + g * 128:c * KO + (g + 1) * 128],
                        start=True,
                        stop=True,
                        tile_position=(r0, 0),
                    )
            srcv = qt[:].rearrange("p (b j n) -> p j b n", b=4, j=4)
            resg = res[:, g * GN:(g + 1) * GN].rearrange(
                "p (i n) -> p i n", i=I
            )
            with nc.allow_low_precision(reason="res fp16"):
                for h in range(2):
                    dstv = (
                        resg[:, 16 * cq + 8 * h:16 * cq + 8 * (h + 1), :]
                        .rearrange("p (j b) n -> p j b n", j=2, b=4)
                    )
                    nc.scalar.copy(dstv, srcv[:, 2 * h:2 * h + 2, :, :])

        # ---- s0 chain (full-depth X.W sum over i) ----
        def emit_s0_chain():
            ps0 = pp.tile([P, 2048], F32, tag="quad")
            for c in range(32):
                nc.tensor.matmul(
                    ps0[:, 0:512],
                    Xt[:, c * 128:(c + 1) * 128],
                    Wr[:, c * KO:(c + 1) * KO],
                    start=(c == 0),
                    stop=(c == 31),
                )
            with nc.allow_low_precision(reason="s0 fp16"):
                nc.scalar.activation(s0h[:], ps0[:, 0:512], COPY, scale=1.0 / I)

        # ---- routing unit pieces ----
        def uv_op(v_h16, g, q):
            UA = rp.tile([P, QI * KC * O], F16, tag="uat", bufs=3, name="UA")
            rg = (
                res[:, g * GN:(g + 1) * GN]
                .rearrange("p (i o k) -> p i o k", i=I, o=O)
                [:, q * QI:(q + 1) * QI, :, :]
                .transpose([0, 2, 1, 3])
            )
            vg = (
                v_h16[:, g * KC * O:(g + 1) * KC * O]
                .rearrange("p (o k) -> p o k", o=O)
                .unsqueeze(2).broadcast_to([P, O, QI, KC])
            )
            nc.vector.tensor_mul(
                UA[:].rearrange("p (o i k) -> p o i k", o=O, i=QI), rg, vg
            )
            return UA

        def ore_op(UA):
            pq = pp.tile([P, 256], F32, tag="pq", bufs=2)
            for o in range(O):
                nc.tensor.matmul(
                    pq[:], E, UA[:, o * 256:(o + 1) * 256],
                    start=(o == 0), stop=(o == O - 1),
                )
            return pq

        def qev_op(pq):
            qt = rp.tile([P, QI * KC], F16, tag="qt", bufs=3)
            with nc.allow_low_precision(reason="q fp16"):
                nc.scalar.copy(qt[:], pq[:])
            return qt

        def ut_op(g, q, qt):
            UT = rp.tile([P, QI * KC * O], F16, tag="uat", bufs=3, name="UT")
            rg = (
                res[:, g * GN:(g + 1) * GN]
                .rearrange("p (i o k) -> p i o k", i=I, o=O)
                [:, q * QI:(q + 1) * QI, :, :]
            )
            qb = (
                qt[:].rearrange("p (i k) -> p i k", i=QI)
                .unsqueeze(2).broadcast_to([P, QI, O, KC])
            )
            nc.vector.tensor_mul(
                UT[:].rearrange("p (i o k) -> p i o k", i=QI, o=O), rg, qb
            )
            return UT

        def yred_op(UT, py, e_acc, q):
            for j in range(QI):
                nc.tensor.matmul(
                    py[:], e_acc, UT[:, j * 128:(j + 1) * 128],
                    start=(q == 0 and j == 0),
                    stop=(q == NQ - 1 and j == QI - 1),
                )

        def ytail_op(py, y_out16, g):
            with nc.allow_low_precision(reason="y fp16"):
                nc.scalar.copy(
                    y_out16[:, g * KC * O:(g + 1) * KC * O], py[:]
                )

        # ---- startup: 4 quads of g0 (DMA-paced), s0 chain, rest of g0 ----
        for cq in range(4):
            produce_quad(0, cq)
        emit_s0_chain()
        for cq in range(4, 8):
            produce_quad(0, cq)

        g32 = sm.tile([P, K], F32, tag="g32")
        g64 = sm.tile([P, K], F32, tag="g64")
        f32_ = sm.tile([P, K], F32, tag="f32_")
        gf = sm.tile([P, K], F32, tag="gf")
        fg = sm.tile([P, K], F32, tag="fg")
        h32 = sm.tile([P, K], F32, tag="h32")
        outf = sm.tile([P, KO], F32, tag="outf")
        s1f = outf
        SC = 64.0

        # 64 global units: (round, group, quarter)
        units = [
            (rnd, g, q) for rnd in range(2) for g in range(NG) for q in range(NQ)
        ]
        # production interleave: 2 quads/unit; g1 -> units 0..3,
        # g2 -> units 4..7, g3 -> units 8..11.
        prod_sched = {}
        for pg in (1, 2, 3):
            for cq in range(8):
                u = (pg - 1) * 4 + cq // 2
                prod_sched.setdefault(u, []).append((pg, cq))

        pys = {}
        pend = []  # (UT, py, g, q, rnd) depth-2 queue

        def flush_one():
            pUT, ppy, pg_, pq_, prnd = pend.pop(0)
            yred_op(pUT, ppy, E64 if prnd == 0 else E, pq_)
            if pq_ == NQ - 1:
                ytail_op(ppy, y1h if prnd == 0 else y2h, pg_)

        with nc.allow_low_precision(reason="fp16 routing"):
            for idx, (rnd, g, q) in enumerate(units):
                if q == 0:
                    pys[(rnd, g)] = pp.tile(
                        [P, 128], F32, tag="py", bufs=2, name="py"
                    )
                py = pys[(rnd, g)]
                UA = uv_op(s0h if rnd == 0 else y1h, g, q)
                pq = ore_op(UA)
                qt = qev_op(pq)
                if rnd == 0:
                    for pg, cq in prod_sched.get(idx, ()):
                        produce_quad(pg, cq)
                if len(pend) >= 2:
                    flush_one()
                UT = ut_op(g, q, qt)
                pend.append((UT, py, g, q, rnd))
                # factor-chain injections (run during round 2's stream)
                if idx == 18:
                    factor(s0h[:], g32, "g")
                    nc.scalar.mul(g64[:], g32[:], SC)
                if idx == 21:
                    # s1 = s0 + g*y1 = s0 + (64 g)*(y1/64)
                    nc.vector.tensor_mul(vKO(s1f), vKO(y1h), bcast_ok(g64))
                    nc.vector.tensor_add(s1f[:], s1f[:], s0h[:])
                    factor(s1f[:], f32_, "f")
                if idx == 25:
                    nc.vector.tensor_add(gf[:], g32[:], f32_[:])
                    nc.scalar.mul(gf[:], gf[:], SC)
                    nc.vector.tensor_mul(fg[:], f32_[:], g32[:])
                    nc.scalar.mul(fg[:], fg[:], SC)
                    nc.vector.tensor_mul(vKO(s2f), vKO(y1h), bcast_ok(gf))
                    nc.vector.tensor_add(s2f[:], s2f[:], s0h[:])
            while pend:
                flush_one()

            # s2 += (64 f g) * (y2/64);  out = factor(s2)*s2
            nc.vector.tensor_mul(vKO(sqb), vKO(y2h), bcast_ok(fg))
            nc.vector.tensor_add(s2f[:], s2f[:], sqb[:])
            factor(s2f[:], h32, "h")
            nc.vector.tensor_mul(vKO(outf), vKO(s2f), bcast_ok(h32))
        nc.sync.dma_start(out_d, outf[:])

    nc.compile()
    return nc


def _host_prep(x, W):
    """x: [B,R,C,I,D] f32; W: [K,I,D,O] f32 -> per-core Xt + shared W_r.

    Xt[(i%4)*32+d, (i//4)*128+p] = x[p, i, d] (d < 16, pad to 32).
    W_r[(i%4)*32+d, (i//4)*512 + g*128 + o*8 + k8] = W[g*8+k8, i, d, o].
    """
    xs = x.reshape(N_CORES, P, I, D)
    a = xs.transpose(0, 2, 3, 1).reshape(N_CORES, 32, 4, D, P)
    ap = np.zeros((N_CORES, 32, 4, D2, P), np.float32)
    ap[:, :, :, 0:D, :] = a
    xt = (
        ap.transpose(0, 2, 3, 1, 4)
        .reshape(N_CORES, 128, 32 * 128)
        .astype(np.float16)
    )
    # (g, o, k8)-ordered W columns
    wko = (
        W.reshape(NG, KC, I, D, O)
        .transpose(2, 3, 0, 4, 1)
        .reshape(I, D, KO)
    )
    b = wko.reshape(32, 4, D, KO)
    bp = np.zeros((32, 4, D2, KO), np.float32)
    bp[:, :, 0:D, :] = b
    wr = bp.transpose(1, 2, 0, 3).reshape(128, 32 * KO).astype(np.float16)
    return xt, wr


def _aux_host():
    e = np.eye(128, dtype=np.float16)
    return np.concatenate([e, e / 64.0], axis=1).astype(np.float16)


def _get_program():
    global _PROGRAM
    if _PROGRAM is None:
        _PROGRAM = _build_program()
    return _PROGRAM


def _in_maps(x, W):
    xt, wr = _host_prep(x, W)
    ei = _aux_host()
    return [
        {"xt": np.ascontiguousarray(xt[c]), "wr": wr, "ei": ei}
        for c in range(N_CORES)
    ]


def _unpack(raw):
    """raw: [P, 512] f32 with cols (g4, o16, k8) -> [2, 8, 8, K, O]."""
    a = raw.reshape(2, 8, 8, NG, O, KC)
    return np.ascontiguousarray(
        a.transpose(0, 1, 2, 3, 5, 4).reshape(2, 8, 8, K, O)
    )


def kernel(**inputs):
    x = np.ascontiguousarray(np.asarray(inputs["inputs"], dtype=np.float32))
    W = np.ascontiguousarray(np.asarray(inputs["W"], dtype=np.float32))
    assert x.shape == (16, 8, 8, 128, 16) and W.shape == (32, 128, 16, 16)

    from concourse.bass_utils import run_bass_kernel_spmd

    nc = _get_program()
    in_maps = _in_maps(x, W)
    r = run_bass_kernel_spmd(nc, in_maps, list(range(N_CORES)))
    outs = [_unpack(r.results[c]["out"]) for c in range(N_CORES)]
    return np.concatenate(outs, axis=0).astype(np.float32)


# revision 26
# speedup vs baseline: 1.6384x; 1.3102x over previous
"""CapsLayer2D dynamic-routing kernel for 8x TRN2 NeuronCores — v6.

Shapes (hardcoded):
  inputs: [B=16, R=8, C=8, I=128, DIN=16] fp32
  W:      [K=32, I=128, DIN=16, DOUT=16] fp32
  out:    [B, R, C, K, DOUT] fp32

Math: 3-round dynamic routing, closed form:
  U[p,k] = res (I x O);  s0 = mean_i U_i;  A = U^T U
  y1 = A s0 = U^T(U s0) ; y2 = A y1
  g = factor(s0); s1 = s0 + g*y1; f = factor(s1)
  out = factor(s2)*s2,  s2 = s0 + (g+f)*y1 + f*g*y2
  factor(s) = (|s|^2/(1+|s|^2)) / sqrt(|s|^2+eps)

v9 design (batch sharded across 8 cores, W replicated):
  W is pre-ordered (g, o, k8) per chunk so production psum columns land
  directly in the res layout: res per k-group g is (i128, o16, k8).
  Both routing contractions run on the PE as identity-stationary matmul
  streams over CONTIGUOUS moving slices (strided moving ops are 2x
  slower on the PE):
    q[i,k] = sum_o uv[o,i,k] -> 16 accumulating mms over 256-col o-slices
    y[o,k] = sum_i ut[i,o,k] -> 32 mms per quarter over 128-col i-slices
  DVE does only the two broadcast multiplies per (group, quarter), all
  in 2x_1p fp16 mode (k8-last layouts incl. the v/q broadcasts). The
  old DVE o-tree (~100us) is gone.

  Pipeline: units = (group, quarter) per round, one-deep software
  pipeline (ut/yred of unit n emitted in unit n+1) so the
  uv->ore->qev->ut chain latency hides under the next uv op.

  Production: [P,2048] psum quads, 16 banded mms each, psum col =
  b*512 + j*128 so 32-row band b writes only psum bank b — bands
  sharing a bank is an NRT_EXEC_UNIT_UNRECOVERABLE crash on HW.
  Quad tag is single-buffered (psum budget: 4 + pq 2 + py 2 = 8 banks);
  routing PE work interleaves between quads to hide the evac waits.
"""

import sys

import numpy as np

sys.path.insert(0, "/opt/trn_rl_repo")

P, I, D, D2, K, O = 128, 128, 16, 32, 32, 16
KC = 8          # k-group size
NG = K // KC    # 4 groups
GN = I * KC * O  # 16384 elements per group block
KO = K * O      # 512
QI = 32         # i's per quarter
NQ = I // QI    # 4 quarters
N_CORES = 8
EPS = 1e-7

_PROGRAM = None


def _build_program():
    from contextlib import ExitStack

    import concourse.tile as tile
    from concourse import bacc, mybir

    F32 = mybir.dt.float32
    F16 = mybir.dt.float16
    ADD = mybir.AluOpType.add
    X = mybir.AxisListType.X
    SQRT = mybir.ActivationFunctionType.Sqrt
    COPY = mybir.ActivationFunctionType.Copy

    nc = bacc.Bacc("TRN2", target_bir_lowering=False, debug=False)

    xt_d = nc.dram_tensor("xt", [P, 32 * 128], F16, kind="ExternalInput").ap()
    wr_d = nc.dram_tensor("wr", [P, 32 * KO], F16, kind="ExternalInput").ap()
    ei_d = nc.dram_tensor("ei", [P, 256], F16, kind="ExternalInput").ap()
    out_d = nc.dram_tensor("out", [P, KO], F32, kind="ExternalOutput").ap()

    with ExitStack() as ctx:
        tc = ctx.enter_context(tile.TileContext(nc))

        pp = ctx.enter_context(tc.tile_pool(name="pp", bufs=1, space="PSUM"))
        rp = ctx.enter_context(tc.tile_pool(name="resp", bufs=1))
        sm = ctx.enter_context(tc.tile_pool(name="small", bufs=1))

        res = rp.tile([P, NG * GN], F16)     # [P, 65536] per-g (i, o, k8)
        AUX = rp.tile([P, 256], F16)         # [E | E/64]
        Xt = rp.tile([P, 32 * 128], F16)
        Wr = rp.tile([P, 32 * KO], F16)      # per-chunk cols (g, o, k8)
        E = AUX[:, 0:128]
        E64 = AUX[:, 128:256]

        # ---- small tiles ----
        s0h = sm.tile([P, KO], F16, tag="s0h")   # (g, o, k8)
        y1h = sm.tile([P, KO], F16, tag="y1h")   # holds y1/64
        y2h = sm.tile([P, KO], F16, tag="y2h")   # holds y2/64
        sqb = sm.tile([P, KO], F32, tag="sqb")
        s2f = sm.tile([P, KO], F32, tag="s2f")
        eps_t = sm.tile([P, 1], F32, tag="eps")
        nc.vector.memset(eps_t[:], EPS)

        def factor(src, out32, tag):
            """out32[p, (g,k8)] = (nsq/(1+nsq))/sqrt(nsq+eps), nsq over o."""
            nc.scalar.square(sqb[:], src)
            nsq = sm.tile([P, K], F32, tag=f"nsq_{tag}")
            nc.vector.tensor_reduce(
                nsq[:].rearrange("p (g k) -> p g k", g=NG),
                sqb[:].rearrange("p (g o k) -> p g k o", g=NG, o=O),
                X, ADD,
            )
            rt = sm.tile([P, K], F32, tag="f_rt")
            nc.scalar.activation(rt[:], nsq[:], SQRT, bias=eps_t[:])
            b1 = sm.tile([P, K], F32, tag="f_b1")
            nc.scalar.add(b1[:], nsq[:], 1.0)
            den = sm.tile([P, K], F32, tag="f_den")
            nc.vector.tensor_mul(den[:], rt[:], b1[:])
            rin = sm.tile([P, K], F32, tag="f_rin")
            nc.vector.reciprocal(rin[:], den[:])
            nc.vector.tensor_mul(out32[:], nsq[:], rin[:])

        def bcast_ok(v32):
            """[p, (g,k8)] -> broadcast view [p, g, o, k8] (k8 last, 2x)."""
            return (
                v32[:].rearrange("p (g k) -> p g k", g=NG)
                .unsqueeze(2).broadcast_to([P, NG, O, KC])
            )

        def vKO(t):
            return t[:].rearrange("p (g o k) -> p g o k", g=NG, o=O)

        nc.sync.dma_start(AUX[:], ei_d)
        for q in range(2):
            nc.sync.dma_start(
                Xt[:, q * 2048:(q + 1) * 2048], xt_d[:, q * 2048:(q + 1) * 2048]
            )
        for q in range(8):
            nc.sync.dma_start(
                Wr[:, q * 2048:(q + 1) * 2048], wr_d[:, q * 2048:(q + 1) * 2048]
            )

        # ---- PE warm-up (clock ramp; overlaps DMA) ----
        q0 = pp.tile([P, 2048], F32, tag="quad")
        for w in range(24):
            nc.tensor.matmul(
                q0[:, 1024:1152],
                Xt[0:32, 0:128],
                Xt[0:32, 0:128],
                start=(w == 0),
                stop=(w == 23),
                tile_position=(0, 0),
            )

        # ---- production: [P,2048] quads, 16 banded mms, bank b = band b --
        # i = 16*cq + 4*j + b ; c = 4*cq + j ; psum col = b*512 + j*128.
        # Evac as two 1024-col halves with dims (j2, b4, n128).
        def produce_quad(g, cq):
            qt = pp.tile([P, 2048], F32, tag="quad")
            for b in range(4):
                r0 = b * 32
                for j in range(4):
                    c = 4 * cq + j
                    nc.tensor.matmul(
                        qt[:, b * 512 + j * 128:b * 512 + (j + 1) * 128],
                        Xt[r0:r0 + 32, c * 128:(c + 1) * 128],
                        Wr[r0:r0 + 32, c * KO + g * 128:c * KO + (g + 1) * 128],
                        start=True,
                        stop=True,
                        tile_position=(r0, 0),
                    )
            srcv = qt[:].rearrange("p (b j n) -> p j b n", b=4, j=4)
            resg = res[:, g * GN:(g + 1) * GN].rearrange(
                "p (i n) -> p i n", i=I
            )
            with nc.allow_low_precision(reason="res fp16"):
                for h in range(2):
                    dstv = (
                        resg[:, 16 * cq + 8 * h:16 * cq + 8 * (h + 1), :]
                        .rearrange("p (j b) n -> p j b n", j=2, b=4)
                    )
                    nc.scalar.copy(dstv, srcv[:, 2 * h:2 * h + 2, :, :])

        # ---- s0 chain (full-depth X.W sum over i) ----
        def emit_s0_chain():
            ps0 = pp.tile([P, 2048], F32, tag="quad")
            for c in range(32):
                nc.tensor.matmul(
                    ps0[:, 0:512],
                    Xt[:, c * 128:(c + 1) * 128],
                    Wr[:, c * KO:(c + 1) * KO],
                    start=(c == 0),
                    stop=(c == 31),
                )
            with nc.allow_low_precision(reason="s0 fp16"):
                nc.scalar.activation(s0h[:], ps0[:, 0:512], COPY, scale=1.0 / I)

        # ---- routing unit pieces ----
        def uv_op(v_h16, g, q):
            UA = rp.tile([P, QI * KC * O], F16, tag="ua", bufs=2, name="UA")
            rg = (
                res[:, g * GN:(g + 1) * GN]
                .rearrange("p (i o k) -> p i o k", i=I, o=O)
                [:, q * QI:(q + 1) * QI, :, :]
                .transpose([0, 2, 1, 3])
            )
            vg = (
                v_h16[:, g * KC * O:(g + 1) * KC * O]
                .rearrange("p (o k) -> p o k", o=O)
                .unsqueeze(2).broadcast_to([P, O, QI, KC])
            )
            nc.vector.tensor_mul(
                UA[:].rearrange("p (o i k) -> p o i k", o=O, i=QI), rg, vg
            )
            return UA

        def ore_op(UA):
            pq = pp.tile([P, 256], F32, tag="pq", bufs=2)
            for o in range(O):
                nc.tensor.matmul(
                    pq[:], E, UA[:, o * 256:(o + 1) * 256],
                    start=(o == 0), stop=(o == O - 1),
                )
            return pq

        def qev_op(pq):
            qt = rp.tile([P, QI * KC], F16, tag="qt", bufs=2)
            with nc.allow_low_precision(reason="q fp16"):
                nc.scalar.copy(qt[:], pq[:])
            return qt

        def ut_op(g, q, qt):
            UT = rp.tile([P, QI * KC * O], F16, tag="ut", bufs=1, name="UT")
            rg = (
                res[:, g * GN:(g + 1) * GN]
                .rearrange("p (i o k) -> p i o k", i=I, o=O)
                [:, q * QI:(q + 1) * QI, :, :]
            )
            qb = (
                qt[:].rearrange("p (i k) -> p i k", i=QI)
                .unsqueeze(2).broadcast_to([P, QI, O, KC])
            )
            nc.vector.tensor_mul(
                UT[:].rearrange("p (i o k) -> p i o k", i=QI, o=O), rg, qb
            )
            return UT

        def yred_op(UT, py, e_acc, q):
            for j in range(QI):
                nc.tensor.matmul(
                    py[:], e_acc, UT[:, j * 128:(j + 1) * 128],
                    start=(q == 0 and j == 0),
                    stop=(q == NQ - 1 and j == QI - 1),
                )

        def ytail_op(py, y_out16, g):
            with nc.allow_low_precision(reason="y fp16"):
                nc.scalar.copy(
                    y_out16[:, g * KC * O:(g + 1) * KC * O], py[:]
                )

        # ---- startup: 4 quads of g0 (DMA-paced), s0 chain, rest of g0 ----
        for cq in range(4):
            produce_quad(0, cq)
        emit_s0_chain()
        for cq in range(4, 8):
            produce_quad(0, cq)

        g32 = sm.tile([P, K], F32, tag="g32")
        g64 = sm.tile([P, K], F32, tag="g64")
        f32_ = sm.tile([P, K], F32, tag="f32_")
        gf = sm.tile([P, K], F32, tag="gf")
        fg = sm.tile([P, K], F32, tag="fg")
        h32 = sm.tile([P, K], F32, tag="h32")
        outf = sm.tile([P, KO], F32, tag="outf")
        s1f = outf
        SC = 64.0

        units = [(g, q) for g in range(NG) for q in range(NQ)]
        # production interleave: 2 quads per unit; g1 -> units 0..3,
        # g2 -> units 4..7, g3 -> units 8..11.
        prod_sched = {}
        for pg in (1, 2, 3):
            for cq in range(8):
                u = (pg - 1) * 4 + cq // 2
                prod_sched.setdefault(u, []).append((pg, cq))

        with nc.allow_low_precision(reason="fp16 routing"):
            for rnd in range(2):
                v_h16 = s0h if rnd == 0 else y1h
                y_out = y1h if rnd == 0 else y2h
                e_acc = E64 if rnd == 0 else E
                pend = None
                py = None
                for idx, (g, q) in enumerate(units):
                    if q == 0:
                        py = pp.tile([P, 128], F32, tag="py", bufs=2)
                    UA = uv_op(v_h16, g, q)
                    pq = ore_op(UA)
                    qt = qev_op(pq)
                    if rnd == 0:
                        for pg, cq in prod_sched.get(idx, ()):
                            produce_quad(pg, cq)
                    if pend is not None:
                        pUT, ppy, pg_, pq_ = pend
                        yred_op(pUT, ppy, e_acc, pq_)
                        if pq_ == NQ - 1:
                            ytail_op(ppy, y_out, pg_)
                    UT = ut_op(g, q, qt)
                    pend = (UT, py, g, q)
                    # inject factor-chain work into round 2's stream
                    if rnd == 1 and idx == 2:
                        factor(s0h[:], g32, "g")
                        nc.scalar.mul(g64[:], g32[:], SC)
                    if rnd == 1 and idx == 5:
                        # s1 = s0 + g*y1 = s0 + (64 g)*(y1/64)
                        nc.vector.tensor_mul(vKO(s1f), vKO(y1h), bcast_ok(g64))
                        nc.vector.tensor_add(s1f[:], s1f[:], s0h[:])
                        factor(s1f[:], f32_, "f")
                    if rnd == 1 and idx == 9:
                        nc.vector.tensor_add(gf[:], g32[:], f32_[:])
                        nc.scalar.mul(gf[:], gf[:], SC)
                        nc.vector.tensor_mul(fg[:], f32_[:], g32[:])
                        nc.scalar.mul(fg[:], fg[:], SC)
                        nc.vector.tensor_mul(vKO(s2f), vKO(y1h), bcast_ok(gf))
                        nc.vector.tensor_add(s2f[:], s2f[:], s0h[:])
                pUT, ppy, pg_, pq_ = pend
                yred_op(pUT, ppy, e_acc, pq_)
                ytail_op(ppy, y_out, pg_)

            # s2 += (64 f g) * (y2/64);  out = factor(s2)*s2
            nc.vector.tensor_mul(vKO(sqb), vKO(y2h), bcast_ok(fg))
            nc.vector.tensor_add(s2f[:], s2f[:], sqb[:])
            factor(s2f[:], h32, "h")
            nc.vector.tensor_mul(vKO(outf), vKO(s2f), bcast_ok(h32))
        nc.sync.dma_start(out_d, outf[:])

    nc.compile()
    return nc


def _host_prep(x, W):
    """x: [B,R,C,I,D] f32; W: [K,I,D,O] f32 -> per-core Xt + shared W_r.

    Xt[(i%4)*32+d, (i//4)*128+p] = x[p, i, d] (d < 16, pad to 32).
    W_r[(i%4)*32+d, (i//4)*512 + g*128 + o*8 + k8] = W[g*8+k8, i, d, o].
    """
    xs = x.reshape(N_CORES, P, I, D)
    a = xs.transpose(0, 2, 3, 1).reshape(N_CORES, 32, 4, D, P)
    ap = np.zeros((N_CORES, 32, 4, D2, P), np.float32)
    ap[:, :, :, 0:D, :] = a
    xt = (
        ap.transpose(0, 2, 3, 1, 4)
        .reshape(N_CORES, 128, 32 * 128)
        .astype(np.float16)
    )
    # (g, o, k8)-ordered W columns
    wko = (
        W.reshape(NG, KC, I, D, O)
        .transpose(2, 3, 0, 4, 1)
        .reshape(I, D, KO)
    )
    b = wko.reshape(32, 4, D, KO)
    bp = np.zeros((32, 4, D2, KO), np.float32)
    bp[:, :, 0:D, :] = b
    wr = bp.transpose(1, 2, 0, 3).reshape(128, 32 * KO).astype(np.float16)
    return xt, wr


def _aux_host():
    e = np.eye(128, dtype=np.float16)
    return np.concatenate([e, e / 64.0], axis=1).astype(np.float16)


def _get_program():
    global _PROGRAM
    if _PROGRAM is None:
        _PROGRAM = _build_program()
    return _PROGRAM


def _in_maps(x, W):
    xt, wr = _host_prep(x, W)
    ei = _aux_host()
    return [
        {"xt": np.ascontiguousarray(xt[c]), "wr": wr, "ei": ei}
        for c in range(N_CORES)
    ]


def _unpack(raw):
    """raw: [P, 512] f32 with cols (g4, o16, k8) -> [2, 8, 8, K, O]."""
    a = raw.reshape(2, 8, 8, NG, O, KC)
    return np.ascontiguousarray(
        a.transpose(0, 1, 2, 3, 5, 4).reshape(2, 8, 8, K, O)
    )


def kernel(**inputs):
    x = np.ascontiguousarray(np.asarray(inputs["inputs"], dtype=np.float32))
    W = np.ascontiguousarray(np.asarray(inputs["W"], dtype=np.float32))
    assert x.shape == (16, 8, 8, 128, 16) and W.shape == (32, 128, 16, 16)

    from concourse.bass_utils import run_bass_kernel_spmd

    nc = _get_program()
    in_maps = _in_maps(x, W)
    r = run_bass_kernel_spmd(nc, in_maps, list(range(N_CORES)))
    outs = [_unpack(r.results[c]["out"]) for c in range(N_CORES)]
    return np.concatenate(outs, axis=0).astype(np.float32)


# revision 28
# speedup vs baseline: 1.6534x; 1.0091x over previous
"""CapsLayer2D dynamic-routing kernel for 8x TRN2 NeuronCores — v6.

Shapes (hardcoded):
  inputs: [B=16, R=8, C=8, I=128, DIN=16] fp32
  W:      [K=32, I=128, DIN=16, DOUT=16] fp32
  out:    [B, R, C, K, DOUT] fp32

Math: 3-round dynamic routing, closed form:
  U[p,k] = res (I x O);  s0 = mean_i U_i;  A = U^T U
  y1 = A s0 = U^T(U s0) ; y2 = A y1
  g = factor(s0); s1 = s0 + g*y1; f = factor(s1)
  out = factor(s2)*s2,  s2 = s0 + (g+f)*y1 + f*g*y2
  factor(s) = (|s|^2/(1+|s|^2)) / sqrt(|s|^2+eps)

v9 design (batch sharded across 8 cores, W replicated):
  W is pre-ordered (g, o, k8) per chunk so production psum columns land
  directly in the res layout: res per k-group g is (i128, o16, k8).
  Both routing contractions run on the PE as identity-stationary matmul
  streams over CONTIGUOUS moving slices (strided moving ops are 2x
  slower on the PE):
    q[i,k] = sum_o uv[o,i,k] -> 16 accumulating mms over 256-col o-slices
    y[o,k] = sum_i ut[i,o,k] -> 32 mms per quarter over 128-col i-slices
  DVE does only the two broadcast multiplies per (group, quarter), all
  in 2x_1p fp16 mode (k8-last layouts incl. the v/q broadcasts). The
  old DVE o-tree (~100us) is gone.

  Pipeline: units = (group, quarter) per round, one-deep software
  pipeline (ut/yred of unit n emitted in unit n+1) so the
  uv->ore->qev->ut chain latency hides under the next uv op.

  Production: [P,2048] psum quads, 16 banded mms each, psum col =
  b*512 + j*128 so 32-row band b writes only psum bank b — bands
  sharing a bank is an NRT_EXEC_UNIT_UNRECOVERABLE crash on HW.
  Quad tag is single-buffered (psum budget: 4 + pq 2 + py 2 = 8 banks);
  routing PE work interleaves between quads to hide the evac waits.
"""

import sys

import numpy as np

sys.path.insert(0, "/opt/trn_rl_repo")

P, I, D, D2, K, O = 128, 128, 16, 32, 32, 16
KC = 8          # k-group size
NG = K // KC    # 4 groups
GN = I * KC * O  # 16384 elements per group block
KO = K * O      # 512
QI = 32         # i's per quarter
NQ = I // QI    # 4 quarters
N_CORES = 8
EPS = 1e-7

_PROGRAM = None


def _build_program():
    from contextlib import ExitStack

    import concourse.tile as tile
    from concourse import bacc, mybir

    F32 = mybir.dt.float32
    F16 = mybir.dt.float16
    ADD = mybir.AluOpType.add
    X = mybir.AxisListType.X
    SQRT = mybir.ActivationFunctionType.Sqrt
    COPY = mybir.ActivationFunctionType.Copy

    nc = bacc.Bacc("TRN2", target_bir_lowering=False, debug=False)

    xt_d = nc.dram_tensor("xt", [P, 32 * 128], F16, kind="ExternalInput").ap()
    wr_d = nc.dram_tensor("wr", [P, 32 * KO], F16, kind="ExternalInput").ap()
    ei_d = nc.dram_tensor("ei", [P, 256], F16, kind="ExternalInput").ap()
    out_d = nc.dram_tensor("out", [P, KO], F32, kind="ExternalOutput").ap()

    with ExitStack() as ctx:
        tc = ctx.enter_context(tile.TileContext(nc))

        pp = ctx.enter_context(tc.tile_pool(name="pp", bufs=1, space="PSUM"))
        rp = ctx.enter_context(tc.tile_pool(name="resp", bufs=1))
        sm = ctx.enter_context(tc.tile_pool(name="small", bufs=1))

        res = rp.tile([P, NG * GN], F16)     # [P, 65536] per-g (i, o, k8)
        AUX = rp.tile([P, 256], F16)         # [E | E/64]
        Xt = rp.tile([P, 32 * 128], F16)
        Wr = rp.tile([P, 32 * KO], F16)      # per-chunk cols (g, o, k8)
        E = AUX[:, 0:128]
        E64 = AUX[:, 128:256]

        # ---- small tiles ----
        s0h = sm.tile([P, KO], F16, tag="s0h")   # (g, o, k8)
        y1h = sm.tile([P, KO], F16, tag="y1h")   # holds y1/64
        y2h = sm.tile([P, KO], F16, tag="y2h")   # holds y2/64
        sqb = sm.tile([P, KO], F32, tag="sqb")
        s2f = sm.tile([P, KO], F32, tag="s2f")
        eps_t = sm.tile([P, 1], F32, tag="eps")
        nc.vector.memset(eps_t[:], EPS)

        def factor(src, out32, tag):
            """out32[p, (g,k8)] = (nsq/(1+nsq))/sqrt(nsq+eps), nsq over o."""
            nc.scalar.square(sqb[:], src)
            nsq = sm.tile([P, K], F32, tag=f"nsq_{tag}")
            nc.vector.tensor_reduce(
                nsq[:].rearrange("p (g k) -> p g k", g=NG),
                sqb[:].rearrange("p (g o k) -> p g k o", g=NG, o=O),
                X, ADD,
            )
            rt = sm.tile([P, K], F32, tag="f_rt")
            nc.scalar.activation(rt[:], nsq[:], SQRT, bias=eps_t[:])
            b1 = sm.tile([P, K], F32, tag="f_b1")
            nc.scalar.add(b1[:], nsq[:], 1.0)
            den = sm.tile([P, K], F32, tag="f_den")
            nc.vector.tensor_mul(den[:], rt[:], b1[:])
            rin = sm.tile([P, K], F32, tag="f_rin")
            nc.vector.reciprocal(rin[:], den[:])
            nc.vector.tensor_mul(out32[:], nsq[:], rin[:])

        def bcast_ok(v32):
            """[p, (g,k8)] -> broadcast view [p, g, o, k8] (k8 last, 2x)."""
            return (
                v32[:].rearrange("p (g k) -> p g k", g=NG)
                .unsqueeze(2).broadcast_to([P, NG, O, KC])
            )

        def vKO(t):
            return t[:].rearrange("p (g o k) -> p g o k", g=NG, o=O)

        nc.sync.dma_start(AUX[:], ei_d)
        for q in range(2):
            nc.sync.dma_start(
                Xt[:, q * 2048:(q + 1) * 2048], xt_d[:, q * 2048:(q + 1) * 2048]
            )
        for q in range(8):
            nc.sync.dma_start(
                Wr[:, q * 2048:(q + 1) * 2048], wr_d[:, q * 2048:(q + 1) * 2048]
            )

        # ---- PE warm-up (clock ramp; overlaps DMA) ----
        q0 = pp.tile([P, 2048], F32, tag="quad")
        for w in range(24):
            nc.tensor.matmul(
                q0[:, 1024:1152],
                Xt[0:32, 0:128],
                Xt[0:32, 0:128],
                start=(w == 0),
                stop=(w == 23),
                tile_position=(0, 0),
            )

        # ---- production: [P,2048] quads, 16 banded mms, bank b = band b --
        # i = 16*cq + 4*j + b ; c = 4*cq + j ; psum col = b*512 + j*128.
        # Evac as two 1024-col halves with dims (j2, b4, n128).
        def produce_quad(g, cq):
            qt = pp.tile([P, 2048], F32, tag="quad")
            for b in range(4):
                r0 = b * 32
                for j in range(4):
                    c = 4 * cq + j
                    nc.tensor.matmul(
                        qt[:, b * 512 + j * 128:b * 512 + (j + 1) * 128],
                        Xt[r0:r0 + 32, c * 128:(c + 1) * 128],
                        Wr[r0:r0 + 32, c * KO + g * 128:c * KO + (g + 1) * 128],
                        start=True,
                        stop=True,
                        tile_position=(r0, 0),
                    )
            srcv = qt[:].rearrange("p (b j n) -> p j b n", b=4, j=4)
            resg = res[:, g * GN:(g + 1) * GN].rearrange(
                "p (i n) -> p i n", i=I
            )
            with nc.allow_low_precision(reason="res fp16"):
                for h in range(2):
                    dstv = (
                        resg[:, 16 * cq + 8 * h:16 * cq + 8 * (h + 1), :]
                        .rearrange("p (j b) n -> p j b n", j=2, b=4)
                    )
                    nc.scalar.copy(dstv, srcv[:, 2 * h:2 * h + 2, :, :])

        # ---- s0 chain (full-depth X.W sum over i) ----
        # On the 'py'-tag psum (1 bank) so its mms interleave between the
        # single-buffered production quads without joining their rotation.
        ps0 = pp.tile([P, 512], F32, tag="py", bufs=2)

        def s0_mms(c0, c1):
            for c in range(c0, c1):
                nc.tensor.matmul(
                    ps0[:],
                    Xt[:, c * 128:(c + 1) * 128],
                    Wr[:, c * KO:(c + 1) * KO],
                    start=(c == 0),
                    stop=(c == 31),
                )

        # ---- routing unit pieces ----
        def uv_op(v_h16, g, q):
            UA = rp.tile([P, QI * KC * O], F16, tag="ua", bufs=2, name="UA")
            rg = (
                res[:, g * GN:(g + 1) * GN]
                .rearrange("p (i o k) -> p i o k", i=I, o=O)
                [:, q * QI:(q + 1) * QI, :, :]
                .transpose([0, 2, 1, 3])
            )
            vg = (
                v_h16[:, g * KC * O:(g + 1) * KC * O]
                .rearrange("p (o k) -> p o k", o=O)
                .unsqueeze(2).broadcast_to([P, O, QI, KC])
            )
            nc.vector.tensor_mul(
                UA[:].rearrange("p (o i k) -> p o i k", o=O, i=QI), rg, vg
            )
            return UA

        def ore_op(UA):
            pq = pp.tile([P, 256], F32, tag="pq", bufs=2)
            for o in range(O):
                nc.tensor.matmul(
                    pq[:], E, UA[:, o * 256:(o + 1) * 256],
                    start=(o == 0), stop=(o == O - 1),
                )
            return pq

        def qev_op(pq):
            qt = rp.tile([P, QI * KC], F16, tag="qt", bufs=2)
            with nc.allow_low_precision(reason="q fp16"):
                nc.scalar.copy(qt[:], pq[:])
            return qt

        def ut_op(g, q, qt):
            UT = rp.tile([P, QI * KC * O], F16, tag="ut", bufs=1, name="UT")
            rg = (
                res[:, g * GN:(g + 1) * GN]
                .rearrange("p (i o k) -> p i o k", i=I, o=O)
                [:, q * QI:(q + 1) * QI, :, :]
            )
            qb = (
                qt[:].rearrange("p (i k) -> p i k", i=QI)
                .unsqueeze(2).broadcast_to([P, QI, O, KC])
            )
            nc.vector.tensor_mul(
                UT[:].rearrange("p (i o k) -> p i o k", i=QI, o=O), rg, qb
            )
            return UT

        def yred_op(UT, py, e_acc, q):
            for j in range(QI):
                nc.tensor.matmul(
                    py[:], e_acc, UT[:, j * 128:(j + 1) * 128],
                    start=(q == 0 and j == 0),
                    stop=(q == NQ - 1 and j == QI - 1),
                )

        def ytail_op(py, y_out16, g):
            with nc.allow_low_precision(reason="y fp16"):
                nc.scalar.copy(
                    y_out16[:, g * KC * O:(g + 1) * KC * O], py[:]
                )

        # ---- startup: g0 quads with s0-chain mms interleaved, so the
        # quad-evac waits (bufs=1) hide under s0 work and s0h lands early.
        produce_quad(0, 0)
        s0_mms(0, 8)
        produce_quad(0, 1)
        s0_mms(8, 16)
        produce_quad(0, 2)
        s0_mms(16, 24)
        produce_quad(0, 3)
        s0_mms(24, 32)
        with nc.allow_low_precision(reason="s0 fp16"):
            nc.scalar.activation(s0h[:], ps0[:], COPY, scale=1.0 / I)
        for cq in range(4, 8):
            produce_quad(0, cq)

        g32 = sm.tile([P, K], F32, tag="g32")
        g64 = sm.tile([P, K], F32, tag="g64")
        f32_ = sm.tile([P, K], F32, tag="f32_")
        gf = sm.tile([P, K], F32, tag="gf")
        fg = sm.tile([P, K], F32, tag="fg")
        h32 = sm.tile([P, K], F32, tag="h32")
        outf = sm.tile([P, KO], F32, tag="outf")
        s1f = outf
        SC = 64.0

        units = [(g, q) for g in range(NG) for q in range(NQ)]
        # production interleave: 2 quads per unit; g1 -> units 0..3,
        # g2 -> units 4..7, g3 -> units 8..11.
        prod_sched = {}
        for pg in (1, 2, 3):
            for cq in range(8):
                u = (pg - 1) * 4 + cq // 2
                prod_sched.setdefault(u, []).append((pg, cq))

        with nc.allow_low_precision(reason="fp16 routing"):
            for rnd in range(2):
                v_h16 = s0h if rnd == 0 else y1h
                y_out = y1h if rnd == 0 else y2h
                e_acc = E64 if rnd == 0 else E
                pend = None
                py = None
                for idx, (g, q) in enumerate(units):
                    if q == 0:
                        py = pp.tile([P, 128], F32, tag="py", bufs=2)
                    UA = uv_op(v_h16, g, q)
                    pq = ore_op(UA)
                    qt = qev_op(pq)
                    if rnd == 0:
                        for pg, cq in prod_sched.get(idx, ()):
                            produce_quad(pg, cq)
                    if pend is not None:
                        pUT, ppy, pg_, pq_ = pend
                        yred_op(pUT, ppy, e_acc, pq_)
                        if pq_ == NQ - 1:
                            ytail_op(ppy, y_out, pg_)
                    UT = ut_op(g, q, qt)
                    pend = (UT, py, g, q)
                    # inject factor-chain work into round 2's stream
                    if rnd == 1 and idx == 2:
                        factor(s0h[:], g32, "g")
                        nc.scalar.mul(g64[:], g32[:], SC)
                    if rnd == 1 and idx == 5:
                        # s1 = s0 + g*y1 = s0 + (64 g)*(y1/64)
                        nc.vector.tensor_mul(vKO(s1f), vKO(y1h), bcast_ok(g64))
                        nc.vector.tensor_add(s1f[:], s1f[:], s0h[:])
                        factor(s1f[:], f32_, "f")
                    if rnd == 1 and idx == 9:
                        nc.vector.tensor_add(gf[:], g32[:], f32_[:])
                        nc.scalar.mul(gf[:], gf[:], SC)
                        nc.vector.tensor_mul(fg[:], f32_[:], g32[:])
                        nc.scalar.mul(fg[:], fg[:], SC)
                        nc.vector.tensor_mul(vKO(s2f), vKO(y1h), bcast_ok(gf))
                        nc.vector.tensor_add(s2f[:], s2f[:], s0h[:])
                pUT, ppy, pg_, pq_ = pend
                yred_op(pUT, ppy, e_acc, pq_)
                ytail_op(ppy, y_out, pg_)

            # s2 += (64 f g) * (y2/64);  out = factor(s2)*s2
            nc.vector.tensor_mul(vKO(sqb), vKO(y2h), bcast_ok(fg))
            nc.vector.tensor_add(s2f[:], s2f[:], sqb[:])
            factor(s2f[:], h32, "h")
            nc.vector.tensor_mul(vKO(outf), vKO(s2f), bcast_ok(h32))
        nc.sync.dma_start(out_d, outf[:])

    nc.compile()
    return nc


def _host_prep(x, W):
    """x: [B,R,C,I,D] f32; W: [K,I,D,O] f32 -> per-core Xt + shared W_r.

    Xt[(i%4)*32+d, (i//4)*128+p] = x[p, i, d] (d < 16, pad to 32).
    W_r[(i%4)*32+d, (i//4)*512 + g*128 + o*8 + k8] = W[g*8+k8, i, d, o].
    """
    xs = x.reshape(N_CORES, P, I, D)
    a = xs.transpose(0, 2, 3, 1).reshape(N_CORES, 32, 4, D, P)
    ap = np.zeros((N_CORES, 32, 4, D2, P), np.float32)
    ap[:, :, :, 0:D, :] = a
    xt = (
        ap.transpose(0, 2, 3, 1, 4)
        .reshape(N_CORES, 128, 32 * 128)
        .astype(np.float16)
    )
    # (g, o, k8)-ordered W columns
    wko = (
        W.reshape(NG, KC, I, D, O)
        .transpose(2, 3, 0, 4, 1)
        .reshape(I, D, KO)
    )
    b = wko.reshape(32, 4, D, KO)
    bp = np.zeros((32, 4, D2, KO), np.float32)
    bp[:, :, 0:D, :] = b
    wr = bp.transpose(1, 2, 0, 3).reshape(128, 32 * KO).astype(np.float16)
    return xt, wr


def _aux_host():
    e = np.eye(128, dtype=np.float16)
    return np.concatenate([e, e / 64.0], axis=1).astype(np.float16)


def _get_program():
    global _PROGRAM
    if _PROGRAM is None:
        _PROGRAM = _build_program()
    return _PROGRAM


def _in_maps(x, W):
    xt, wr = _host_prep(x, W)
    ei = _aux_host()
    return [
        {"xt": np.ascontiguousarray(xt[c]), "wr": wr, "ei": ei}
        for c in range(N_CORES)
    ]


def _unpack(raw):
    """raw: [P, 512] f32 with cols (g4, o16, k8) -> [2, 8, 8, K, O]."""
    a = raw.reshape(2, 8, 8, NG, O, KC)
    return np.ascontiguousarray(
        a.transpose(0, 1, 2, 3, 5, 4).reshape(2, 8, 8, K, O)
    )


def kernel(**inputs):
    x = np.ascontiguousarray(np.asarray(inputs["inputs"], dtype=np.float32))
    W = np.ascontiguousarray(np.asarray(inputs["W"], dtype=np.float32))
    assert x.shape == (16, 8, 8, 128, 16) and W.shape == (32, 128, 16, 16)

    from concourse.bass_utils import run_bass_kernel_spmd

    nc = _get_program()
    in_maps = _in_maps(x, W)
    r = run_bass_kernel_spmd(nc, in_maps, list(range(N_CORES)))
    outs = [_unpack(r.results[c]["out"]) for c in range(N_CORES)]
    return np.concatenate(outs, axis=0).astype(np.float32)
